# revision 2
# baseline (speedup 1.0000x reference)
"""Binary residual block (sign-conv x3) on 8 TRN2 NeuronCores.

Data-parallel: batch 64 is split 8 ways (8 images per core); binarized
weights are replicated. Per core the three convs run as PE matmuls with
input channels on the partition (contraction) dim:

  conv1 3x3/s2: x is split into two fp16 limbs (hi = fp16(x),
    lo = fp16(x - hi)); +-1 weights are exact in fp16, so accumulating
    both limb matmuls in fp32 PSUM reproduces fp32 accuracy at full PE
    rate (fp32 matmul would run at 1/4 rate).
  conv2 3x3/s1: inputs are sign() outputs, exactly representable in
    fp8e4, so it runs as fp8 DoubleRow matmuls (256-deep contraction per
    instruction) with bit-exact integer results. HW-measured: the
    256-column DoubleRow LDWEIGHTS pipelines fully behind the 420-column
    matmul stream (173.6 ns/MM ~= pure streaming).
  shortcut 1x1/s2: a single fp32r matmul per PSUM group. fp32r rounds
    operands to a 15-bit significand and then accumulates exactly; +-1
    weights are exact, so the only error is x's rounding (~2^-16
    relative), which feeds the final sign directly (no conv2
    amplification): ~tens of flipped outputs out of 12.8M, far inside
    the 2e-2 rel-err budget. Replaces two fp16 limb matmuls.

Layouts: x limbs live in parity-quadrant form Q[c, h2, w2, h, w] =
xpad[c, 2h+h2, 2w+w2] (29x30 per quadrant) so every stride-2 tap reads
unit-stride columns; sign1 lives zero-padded 30x30 per channel-tile with
a 912-byte tile stride (DoubleRow requires the K-pair stride % 16 == 0).
conv2 streams contiguous 420-lane runs (14 rows x 30 cols incl. pad);
the two pad lanes per row are junk and never read back. The shortcut
reads a separate [28, 30] f32r tile of the even/even quadrant. Each
conv output quarter is one PSUM accumulation group (conv2 + shortcut
share a group); Sign applies on the scalar engine straight out of PSUM.

Latency structure: x arrives as two row-parity DMAs so limb prep starts
at half-transfer; weight DMAs ride the Activation queue in parallel
with x's SP-queue DMAs (weights travel as bf16 = sign-exact, half the
bytes; sign() itself still runs on device); limb subtractions run on
GpSimd so the DVE only does the hi-copies; each output quarter DMAs out
right after its sign. Weights are pre-transposed on the host to the
lhsT layouts the PE wants (layout + sign-preserving downcast only; the
sign() math runs on device). Padded tiles are persistent: the zero ring
is written once, per-image ops only touch the interior.
"""

import numpy as np
import ml_dtypes

P = 128
H = W = 56
OH = OW = 28
H2P = 30        # zero-padded sign1 edge (28 + 2)
QE = 29         # quadrant rows
QW = 30         # quadrant row pitch (28 valid + pad)
N_CORES = 8
IMG = 8         # images per core
NBUF = 3        # persistent tile sets (pipeline depth across images)

_CACHE = {}


def _build(n_cores=N_CORES, img=IMG, repeat=1):
    import concourse.bass as bass  # noqa: F401
    import concourse.tile as tile
    from concourse import bacc, mybir

    AF = mybir.ActivationFunctionType
    f32 = mybir.dt.float32
    f32r = mybir.dt.float32r
    bf16 = mybir.dt.bfloat16
    f16 = mybir.dt.float16
    f8 = mybir.dt.float8e4
    DRPM = mybir.MatmulPerfMode.DoubleRow

    nc = bacc.Bacc("TRN2", target_bir_lowering=False, debug=False,
                   num_devices=n_cores)
    x_d = nc.dram_tensor("x", [img, 128, H, W], f32, kind="ExternalInput")
    # host-pretransposed lhsT layouts (see prep_weights)
    w1_d = nc.dram_tensor("w1", [P, 9, 2, P], bf16, kind="ExternalInput")
    w2_d = nc.dram_tensor("w2", [P, 9, 2, 2, P], bf16, kind="ExternalInput")
    wsc_d = nc.dram_tensor("wsc", [P, 2, P], bf16, kind="ExternalInput")
    y_d = nc.dram_tensor("y", [img, 256, OH, OW], f32, kind="ExternalOutput")

    with tile.TileContext(nc) as tc:
        with (
            tc.tile_pool(name="wpool", bufs=1) as wpool,
            tc.tile_pool(name="xper", bufs=1) as xper,
            tc.tile_pool(name="xin", bufs=3) as xin_pool,
            tc.tile_pool(name="opool", bufs=2) as opool,
            tc.tile_pool(name="wstage", bufs=1) as wstage,
            tc.tile_pool(name="pc1", bufs=4, space="PSUM") as pc1,
            tc.tile_pool(name="pc2", bufs=4, space="PSUM") as pc2,
        ):
            # persistent parity-quadrant limb tiles, sign1 tiles, and
            # f32r shortcut tiles; zero ring written once, interiors
            # rewritten per image
            xhi = [xper.tile([P, 2, 2, QE, QW], f16, tag=f"xhi{j}",
                             name=f"xhi{j}") for j in range(NBUF)]
            xlo = [xper.tile([P, 2, 2, QE, QW], f16, tag=f"xlo{j}",
                             name=f"xlo{j}") for j in range(NBUF)]
            s1b = [xper.tile([P, 2, 912], f8, tag=f"s1{j}",
                             name=f"s1{j}") for j in range(NBUF)]
            xsc = [xper.tile([P, OH, QW], f32r, tag=f"xsc{j}",
                             name=f"xsc{j}") for j in range(NBUF)]
            for t in xhi + xlo + s1b:
                nc.gpsimd.memset(t[:], 0.0)
            for t in xsc:
                nc.gpsimd.memset(t[:].bitcast(f32), 0.0)

            w1t = wpool.tile([P, 9, 2, P], f16, tag="w1t")
            w2t = wpool.tile([P, 9, 2, 2, P], f8, tag="w2t")
            wsctr = wpool.tile([P, 2, P], f32r, tag="wsctr")

            def prep_w():
                # Activation-queue DMAs run parallel to x's SP-queue
                # DMAs; w1 first (needed by conv1 of image 0), w2 last
                # (first needed ~25us in).
                w1s = wstage.tile([P, 9, 2, P], bf16, tag="w1s")
                nc.scalar.dma_start(w1s[:], w1_d[:])
                nc.scalar.activation(w1t[:], w1s[:], AF.Sign)
                wscs = wstage.tile([P, 2, P], bf16, tag="wscs")
                nc.scalar.dma_start(wscs[:], wsc_d[:])
                wsc32 = wstage.tile([P, 2, P], f32, tag="wsc32")
                nc.scalar.activation(wsc32[:], wscs[:], AF.Sign)
                nc.vector.tensor_copy(wsctr[:], wsc32[:])
                w2s = wstage.tile([P, 9, 2, 2, P], bf16, tag="w2s")
                nc.scalar.dma_start(w2s[:], w2_d[:])
                nc.scalar.activation(w2t[:], w2s[:], AF.Sign)

            def load(i):
                hi, lo, sc = xhi[i % NBUF], xlo[i % NBUF], xsc[i % NBUF]
                xv = x_d[i].rearrange("c (h p) w -> c p h w", p=2)
                xa = xin_pool.tile([P, OH, W], f32, tag="xa")  # odd rows
                nc.sync.dma_start(xa[:], xv[:, 1])
                xb = xin_pool.tile([P, OH, W], f32, tag="xb")  # even rows
                nc.sync.dma_start(xb[:], xv[:, 0])
                # quadrant (h2, w2) holds x[2h+1-h2, 2w+1-w2]
                for h2, src in ((0, xa), (1, xb)):
                    srcv = src[:].rearrange("c h (w q) -> c q h w", q=2)
                    for w2 in range(2):
                        dst = (slice(None), h2, w2,
                               slice(1 - h2, 29 - h2), slice(1 - w2, 29 - w2))
                        srcq = srcv[:, 1 - w2]
                        nc.vector.tensor_copy(hi[dst], srcq)
                        nc.gpsimd.tensor_sub(lo[dst], srcq, hi[dst])
                # shortcut reads x[2h, 2w] (even/even), f32r-rounded
                nc.vector.tensor_copy(
                    sc[:, :, 0:OW],
                    xb[:].rearrange("c h (w q) -> c q h w", q=2)[:, 0])
                return hi, lo, sc

            def conv1(i, hi, lo):
                s1 = s1b[i % NBUF]
                for ko in range(2):
                    for hf in range(2):
                        p1 = pc1.tile([P, 14, OW], f32, tag="p1")
                        cnt = 0
                        for limb in (hi, lo):
                            for kh in range(3):
                                for kw in range(3):
                                    rhs = limb[:, kh % 2, kw % 2,
                                               kh // 2 + 14 * hf:
                                               kh // 2 + 14 * hf + 14,
                                               kw // 2: kw // 2 + OW]
                                    nc.tensor.matmul(
                                        p1[:], w1t[:, kh * 3 + kw, ko, :], rhs,
                                        start=(cnt == 0), stop=(cnt == 17))
                                    cnt += 1
                        s1v = s1[:, :, :900].rearrange(
                            "c t (h w) -> c t h w", h=H2P)
                        nc.scalar.activation(
                            s1v[:, ko, 1 + 14 * hf: 15 + 14 * hf, 1:29],
                            p1[:], AF.Sign)
                return s1

            def conv2_out(i, s1, sc):
                ou = opool.tile([P, 2, OH, OW], f32, tag="ou")
                yv = y_d[i].rearrange("(ko m) h w -> m ko h w", ko=2)
                scf = sc[:].rearrange("c h w -> c (h w)")
                for ko in range(2):
                    for hf in range(2):
                        # 9 DoubleRow MMs over contiguous 420-lane runs
                        # (14 rows x 30 incl. pad cols) + 1 fp32r
                        # shortcut MM; lanes with ow in {28, 29} are
                        # junk and never read.
                        p2 = pc2.tile([P, 420], f32, tag="p2")
                        p2v = p2[:].rearrange("c (h w) -> c h w", h=14)
                        for kk in range(9):
                            kh, kw = divmod(kk, 3)
                            base = (kh + 14 * hf) * H2P + kw
                            nc.tensor.matmul(
                                p2[:], w2t[:, kk, ko], s1[:, :, base: base + 420],
                                start=(kk == 0), stop=False,
                                perf_mode=DRPM)
                        nc.tensor.matmul(
                            p2[:], wsctr[:, ko, :],
                            scf[:, 14 * hf * QW: 14 * hf * QW + 420],
                            start=False, stop=True)
                        nc.scalar.activation(
                            ou[:, ko, 14 * hf: 14 * hf + 14, :],
                            p2v[:, :, 0:OW], AF.Sign)
                        nc.sync.dma_start(
                            yv[:, ko, 14 * hf: 14 * hf + 14, :],
                            ou[:, ko, 14 * hf: 14 * hf + 14, :])

            def whole_pass():
                first = load(0)
                prep_w()
                prev = None
                for i in range(img):
                    hi, lo, sc = first if i == 0 else load(i)
                    s1 = conv1(i, hi, lo)
                    if prev is not None:
                        conv2_out(*prev)
                    prev = (i, s1, sc)
                conv2_out(*prev)

            if repeat == 1:
                whole_pass()
            else:
                with tc.For_i(0, repeat, 1):
                    whole_pass()

    nc.compile()
    return nc


def _get_nc(repeat=1):
    if repeat not in _CACHE:
        _CACHE[repeat] = _build(repeat=repeat)
    return _CACHE[repeat]


def prep_weights(w1, w2, w_sc):
    """Host-side lhsT layout prep: pure transposition plus a
    sign-preserving bf16 downcast for transport (bf16 keeps fp32's full
    exponent range, so sign(bf16(w)) == sign(w); sign() itself runs on
    device)."""
    w1 = np.asarray(w1, dtype=np.float32)
    w2 = np.asarray(w2, dtype=np.float32)
    w_sc = np.asarray(w_sc, dtype=np.float32)
    # [c, kh*kw, ko, m] from (K=ko*128+m, c, kh, kw)
    w1t = np.ascontiguousarray(
        w1.transpose(1, 2, 3, 0).reshape(P, 9, 2, P)
    ).astype(ml_dtypes.bfloat16)
    # [cp, kh*kw, ko, ct, m] from (K, C=ct*128+cp, kh, kw)
    w2t = np.ascontiguousarray(
        w2.reshape(2, P, 2, P, 3, 3)           # ko m ct cp kh kw
        .transpose(3, 4, 5, 0, 2, 1)           # cp kh kw ko ct m
        .reshape(P, 9, 2, 2, P)
    ).astype(ml_dtypes.bfloat16)
    wsct = np.ascontiguousarray(
        w_sc[:, :, 0, 0].transpose(1, 0).reshape(P, 2, P)
    ).astype(ml_dtypes.bfloat16)
    return w1t, w2t, wsct


def kernel(x, w1, w2, w_sc):
    from concourse import bass_utils

    x = np.ascontiguousarray(np.asarray(x, dtype=np.float32))
    w1t, w2t, wsct = prep_weights(w1, w2, w_sc)

    nc = _get_nc()
    in_maps = [
        {"x": x[c * IMG:(c + 1) * IMG], "w1": w1t, "w2": w2t, "wsc": wsct}
        for c in range(N_CORES)
    ]
    res = bass_utils.run_bass_kernel_spmd(
        nc, in_maps, core_ids=list(range(N_CORES)))
    y = np.concatenate([res.results[c]["y"] for c in range(N_CORES)], axis=0)
    return y
